# revision 1
# baseline (speedup 1.0000x reference)
"""Causal self-attention (B=4, T=2048, C=1024, H=16, D=64) on 8 TRN2 NeuronCores.

Sharding: core = (batch b, head-group g) with b = core // 2, g = core % 2.
Each core computes heads [8g, 8g+8) of batch b and produces the partial
out-projection (C, T) for its head group; the host sums the two head-group
partials per batch and adds the output bias.

On-device layout notes:
- All activations/weights enter the PE as fp16; PSUM accumulates fp32.
- q/k are produced "transposed" (feature on partitions, t on free dim) so
  scores can be computed as ST[s, t] = k^T q with no transposes anywhere.
- RoPE feature permutation per head: rows [e0..e15, o0..o15, e16..e31,
  o16..o31] (e=even/cos-lane of pair i, o=odd). The pair swap is then a
  16-row swap inside each 32-partition quadrant -> one DVE stream_shuffle.
- Softmax runs unnormalized in the (s, t) orientation: E = exp(S/8); the
  per-t denominator is produced by an extra all-ones column appended to V
  (M=65 in the att@V matmul); normalization divides at the end.
- Causal masking: fully-masked (s, t) tiles are skipped; diagonal tiles are
  zeroed elementwise post-exp with gpsimd.affine_select.
"""

import numpy as np

B, T, C = 4, 2048, 1024
H, D = 16, 64
N_CORES = 8
HPG = H // 2            # heads per core (group)
NCHUNK = 4              # head-pair chunks per core
KT = 8                  # k-tiles of 128 over C
KT_AUG = 9              # + bias/ones k-tile
TT = 4                  # t-tiles of 512 over T
NT = 512                # t tile (matmul N)
VS = 66                 # v column stride per head (64 dims + ones + pad)
VW = HPG * VS           # 528 v columns per k-chunk block
ROPE_BASE = 10000.0

_CACHE = {}


def _d_of_r(r):
    # row r (0..63) inside a head's 64 rotated rows -> original head dim d
    f = (r // 32) * 16 + (r % 16)
    return 2 * f + (1 if (r % 32) >= 16 else 0)


def _f_of_p(p):
    # partition p (0..127) -> rope frequency index
    return ((p // 32) % 2) * 16 + (p % 16)


def _build_nc():
    import concourse.bass as bass  # noqa: F401
    import concourse.tile as tile
    from concourse import bacc, mybir
    from contextlib import ExitStack

    f16 = mybir.dt.float16
    f32 = mybir.dt.float32

    nc = bacc.Bacc(
        "TRN2",
        target_bir_lowering=False,
        debug=False,
        enable_asserts=True,
        num_devices=N_CORES,
    )

    xt_d = nc.dram_tensor("xt", (KT_AUG * 128, T), f16, kind="ExternalInput").ap()
    wqk_d = nc.dram_tensor("wqk", (128, KT * 1024), f16, kind="ExternalInput").ap()
    wv_d = nc.dram_tensor("wv", (128, KT_AUG * VW), f16, kind="ExternalInput").ap()
    wo_d = nc.dram_tensor("wo", (128, NCHUNK * 1024), f16, kind="ExternalInput").ap()
    bqk_d = nc.dram_tensor("bqk", (128, 16), f32, kind="ExternalInput").ap()
    cs_d = nc.dram_tensor("cs", (128, T), f16, kind="ExternalInput").ap()
    css_d = nc.dram_tensor("css", (128, T), f16, kind="ExternalInput").ap()
    ot_d = nc.dram_tensor("ot", (1024, T), f32, kind="ExternalOutput").ap()

    SHUF = list(range(16, 32)) + list(range(0, 16))

    with tile.TileContext(nc) as tc:
        with ExitStack() as ctx, nc.allow_low_precision("fp16 attention pipeline"):
            consts = ctx.enter_context(tc.tile_pool(name="consts", bufs=1))
            qk_pool = ctx.enter_context(tc.tile_pool(name="qk", bufs=2))
            rtmp = ctx.enter_context(tc.tile_pool(name="rtmp", bufs=4))
            e_pool = ctx.enter_context(tc.tile_pool(name="e", bufs=8))
            small = ctx.enter_context(tc.tile_pool(name="small", bufs=3))
            osb = ctx.enter_context(tc.tile_pool(name="osb", bufs=6))
            ps_big = ctx.enter_context(tc.tile_pool(name="psbig", bufs=2, space="PSUM"))
            ps_s = ctx.enter_context(tc.tile_pool(name="pss", bufs=2, space="PSUM"))
            ps_y = ctx.enter_context(tc.tile_pool(name="psy", bufs=1, space="PSUM"))

            # ---- resident tiles + input DMA ----
            xt = consts.tile([128, KT_AUG * T], f16)
            for kc in range(KT_AUG):
                for i in range(2):
                    nc.sync.dma_start(xt[:, kc * T + i * 1024: kc * T + (i + 1) * 1024],
                                      xt_d[kc * 128:(kc + 1) * 128, i * 1024:(i + 1) * 1024])
            def dma_split(dst, src, width, parts):
                step = width // parts
                for i in range(parts):
                    nc.sync.dma_start(dst[:, i * step:(i + 1) * step],
                                      src[:, i * step:(i + 1) * step])

            wqk = consts.tile([128, KT * 1024], f16)
            dma_split(wqk, wqk_d, KT * 1024, 8)
            wv = consts.tile([128, KT_AUG * VW], f16)
            dma_split(wv, wv_d, KT_AUG * VW, 4)
            wo = consts.tile([128, NCHUNK * 1024], f16)
            dma_split(wo, wo_d, NCHUNK * 1024, 4)
            bqk = consts.tile([128, 16], f32)
            nc.sync.dma_start(bqk[:], bqk_d[:])
            cs = consts.tile([128, T], f16)
            dma_split(cs, cs_d, T, 2)
            css = consts.tile([128, T], f16)
            dma_split(css, css_d, T, 2)
            v_sb = consts.tile([128, 16 * VW], f16)
            y_all = consts.tile([128, NCHUNK * T], f16)

            # ---- phase 0: V projection for all 8 heads ----
            with nc.named_scope("vproj"):
                for m in range(16):  # 128-row t-slices
                    psa = ps_big.tile([128, 512], f32, tag="big")
                    psb = ps_s.tile([128, 1024], f32, tag="s")
                    for kc in range(KT_AUG):
                        lhs = xt[:, kc * T + m * 128: kc * T + (m + 1) * 128]
                        nc.tensor.matmul(psa[:], lhs, wv[:, kc * VW: kc * VW + 512],
                                         start=(kc == 0), stop=(kc == KT_AUG - 1))
                        nc.tensor.matmul(psb[:, 0:16], lhs, wv[:, kc * VW + 512: (kc + 1) * VW],
                                         start=(kc == 0), stop=(kc == KT_AUG - 1))
                    nc.vector.tensor_copy(v_sb[:, m * VW: m * VW + 512], psa[:])
                    nc.vector.tensor_copy(v_sb[:, m * VW + 512: (m + 1) * VW], psb[:, 0:16])

            for c in range(NCHUNK):
                # ---- phase 1: q/k projection + RoPE for heads (2c, 2c+1) ----
                rq = qk_pool.tile([128, T], f16, tag="rq")
                rk = qk_pool.tile([128, T], f16, tag="rk")
                with nc.named_scope("qkrope"):
                    for tt in range(TT):
                        t0 = tt * NT
                        for which, dst in ((0, rq), (1, rk)):
                            ps = ps_big.tile([128, 512], f32, tag="big")
                            for kc in range(KT):
                                lhsT = wqk[:, kc * 1024 + c * 256 + which * 128: kc * 1024 + c * 256 + which * 128 + 128]
                                rhs = xt[:, kc * T + t0: kc * T + t0 + NT]
                                nc.tensor.matmul(ps[:], lhsT, rhs, start=(kc == 0), stop=(kc == KT - 1))
                            bcol = bqk[:, c * 4 + which * 2: c * 4 + which * 2 + 1]
                            bswp = bqk[:, c * 4 + which * 2 + 1: c * 4 + which * 2 + 2]
                            s_t = rtmp.tile([128, 512], f32, tag="s")
                            nc.vector.stream_shuffle(s_t[:], ps[:], SHUF)
                            x1 = rtmp.tile([128, 512], f16, tag="x1")
                            nc.vector.scalar_tensor_tensor(
                                out=x1[:], in0=ps[:], scalar=bcol, in1=cs[:, t0:t0 + NT],
                                op0=mybir.AluOpType.add, op1=mybir.AluOpType.mult)
                            x2 = rtmp.tile([128, 512], f16, tag="x2")
                            nc.vector.scalar_tensor_tensor(
                                out=x2[:], in0=s_t[:], scalar=bswp, in1=css[:, t0:t0 + NT],
                                op0=mybir.AluOpType.add, op1=mybir.AluOpType.mult)
                            nc.vector.tensor_add(dst[:, t0:t0 + NT], x1[:], x2[:])

                # ---- phase 2: attention for this chunk ----
                # Both heads of the chunk share 1024-wide paired tiles:
                # cols [0:512) = head 2c, [512:1024) = head 2c+1.
                with nc.named_scope("attn"):
                    for tt in range(TT):
                        t0 = tt * NT
                        sc_max = (t0 + NT) // 128
                        yp = ps_y.tile([65, 1024], f32, tag="y")
                        for sc in range(sc_max):
                            s0 = sc * 128
                            dlt = max(0, s0 - t0)  # first causal-valid col in tile
                            w = NT - dlt
                            sp = ps_s.tile([128, 1024], f32, tag="s")
                            nc.tensor.matmul(sp[:, dlt:NT], rk[0:64, s0:s0 + 128],
                                             rq[0:64, t0 + dlt:t0 + NT],
                                             start=True, stop=True, tile_position=(0, 0))
                            nc.tensor.matmul(sp[:, NT + dlt:2 * NT], rk[64:128, s0:s0 + 128],
                                             rq[64:128, t0 + dlt:t0 + NT],
                                             start=True, stop=True, tile_position=(64, 0))
                            e_t = e_pool.tile([128, 1024], f16)
                            s3 = sp[:].rearrange("p (a b) -> p a b", a=2)[:, :, dlt:]
                            e3 = e_t[:].rearrange("p (a b) -> p a b", a=2)[:, :, dlt:]
                            nc.scalar.activation(e3, s3, mybir.ActivationFunctionType.Exp,
                                                 bias=0.0, scale=0.125)
                            if s0 + 127 > t0:
                                # keep iff j' >= p  (j' is offset within the
                                # shrunken width; diagonal starts at col dlt)
                                nc.gpsimd.affine_select(
                                    out=e3, in_=e3,
                                    compare_op=mybir.AluOpType.is_ge,
                                    fill=0.0, base=0,
                                    pattern=[[0, 2], [1, w]], channel_multiplier=-1)
                            for h in range(2):
                                vcol = sc * VW + VS * (2 * c + h)
                                nc.tensor.matmul(yp[:, h * NT + dlt:(h + 1) * NT],
                                                 v_sb[:, vcol: vcol + 65],
                                                 e_t[:, h * NT + dlt:(h + 1) * NT],
                                                 start=(sc == 0), stop=(sc == sc_max - 1),
                                                 skip_group_check=True)
                        # single read of yp frees its PSUM slot immediately;
                        # normalization then runs SBUF-only (2x DVE mode)
                        yc = small.tile([65, 1024], f32, tag="yc")
                        nc.vector.tensor_copy(yc[:], yp[:])
                        rd = small.tile([1, 1024], f32, tag="rd")
                        nc.vector.reciprocal(rd[:], yc[64:65, :])
                        rbc = small.tile([64, 1024], f32, tag="rbc")
                        nc.gpsimd.partition_broadcast(rbc[:], rd[:])
                        for h in range(2):
                            nc.vector.tensor_mul(
                                y_all[h * 64:(h + 1) * 64, c * T + t0: c * T + t0 + NT],
                                yc[0:64, h * NT:(h + 1) * NT],
                                rbc[:, h * NT:(h + 1) * NT])

            # ---- phase 3: output projection (partial over this core's heads) ----
            with nc.named_scope("oproj"):
                for ct in range(8):
                    for tt in range(TT):
                        t0 = tt * NT
                        po = ps_big.tile([128, 512], f32, tag="big")
                        for c in range(NCHUNK):
                            nc.tensor.matmul(po[:], wo[:, c * 1024 + ct * 128: c * 1024 + ct * 128 + 128],
                                             y_all[:, c * T + t0: c * T + t0 + NT],
                                             start=(c == 0), stop=(c == NCHUNK - 1))
                        ob = osb.tile([128, 512], f32)
                        nc.vector.tensor_copy(ob[:], po[:])
                        nc.sync.dma_start(ot_d[ct * 128:(ct + 1) * 128, t0:t0 + NT], ob[:])

    nc.compile()
    return nc


def _prep_inputs(x, qkv_w, qkv_b):
    """Build the 8 per-core input maps (all host-side numpy)."""
    x = np.asarray(x, dtype=np.float32)
    qkv_w = np.asarray(qkv_w, dtype=np.float32)
    qkv_b = np.asarray(qkv_b, dtype=np.float32)

    # xt per batch: (KT_AUG*128, T) fp16 with row 1024 = ones, rest of aug block 0
    xts = []
    for b in range(B):
        xa = np.zeros((KT_AUG * 128, T), dtype=np.float16)
        xa[:C] = x[b].T.astype(np.float16)
        xa[C] = 1.0
        xts.append(xa)

    r = np.arange(64)
    d_r = 2 * ((r // 32) * 16 + (r % 16)) + ((r % 32) >= 16)  # row -> head dim
    p = np.arange(128)
    f_p = ((p // 32) % 2) * 16 + (p % 16)

    ins_g = []
    for g in range(2):
        # wqk: [p, kc*1024 + c*256 + which*128 + m]
        wqk = np.empty((128, KT * 1024), dtype=np.float16)
        bqk = np.empty((128, 16), dtype=np.float32)
        for c in range(NCHUNK):
            for which in range(2):  # 0=q, 1=k
                rows = np.concatenate([
                    which * C + (8 * g + 2 * c + hh) * 64 + d_r for hh in range(2)
                ])  # 128 feature rows
                blk = qkv_w[rows, :]          # (128 feat, 1024 k)
                for kc in range(KT):
                    wqk[:, kc * 1024 + c * 256 + which * 128:
                        kc * 1024 + c * 256 + which * 128 + 128] = \
                        blk[:, kc * 128:(kc + 1) * 128].T.astype(np.float16)
                bc = qkv_b[rows].astype(np.float32)
                bqk[:, c * 4 + which * 2] = bc
                bqk[:, c * 4 + which * 2 + 1] = bc[p ^ 16]
        # wv: [p, kc*VW + col], col = VS*h + j
        wva = np.zeros((KT_AUG * 128, VW), dtype=np.float32)
        for h in range(HPG):
            rows = 2 * C + (8 * g + h) * 64 + np.arange(64)
            wva[:C, VS * h: VS * h + 64] = qkv_w[rows, :].T
            wva[C, VS * h: VS * h + 64] = qkv_b[rows]
            wva[C, VS * h + 64] = 1.0
        wv = np.empty((128, KT_AUG * VW), dtype=np.float16)
        for kc in range(KT_AUG):
            wv[:, kc * VW:(kc + 1) * VW] = wva[kc * 128:(kc + 1) * 128].astype(np.float16)
        ins_g.append((wqk, bqk, wv))

    # rope tables
    inv_freq = (1.0 / (ROPE_BASE ** (np.arange(0, D, 2) / D))).astype(np.float64)
    t = np.arange(T, dtype=np.float64)
    ang = t[None, :] * inv_freq[f_p][:, None]          # (128, T)
    cs = np.cos(ang).astype(np.float16)
    sgn = np.where((p % 32) < 16, -1.0, 1.0)[:, None]
    css = (sgn * np.sin(ang)).astype(np.float16)

    return xts, ins_g, cs, css


def _prep_wo(out_w, g):
    out_w = np.asarray(out_w, dtype=np.float32)
    wo = np.empty((128, NCHUNK * 1024), dtype=np.float16)
    for c in range(NCHUNK):
        rows = np.concatenate([(8 * g + 2 * c + hh) * 64 + np.arange(64) for hh in range(2)])
        wo[:, c * 1024:(c + 1) * 1024] = out_w[:, rows].astype(np.float16).T
    return wo


def kernel(x, qkv_w, qkv_b, out_w, out_b):
    from concourse.bass_utils import run_bass_kernel_spmd

    if "nc" not in _CACHE:
        _CACHE["nc"] = _build_nc()
    nc = _CACHE["nc"]

    xts, ins_g, cs, css = _prep_inputs(x, qkv_w, qkv_b)
    wos = [_prep_wo(out_w, g) for g in range(2)]
    out_b = np.asarray(out_b, dtype=np.float32)

    in_maps = []
    for core in range(N_CORES):
        b, g = core // 2, core % 2
        wqk, bqk, wv = ins_g[g]
        in_maps.append({
            "xt": xts[b], "wqk": wqk, "wv": wv, "wo": wos[g],
            "bqk": bqk, "cs": cs, "css": css,
        })

    try:
        res = run_bass_kernel_spmd(nc, in_maps, core_ids=list(range(N_CORES)))
    except ModuleNotFoundError:
        # BASS_TRACE set but the NTFF profile hook isn't importable here
        import os
        os.environ["BASS_NEVER_TRACE"] = "1"
        res = run_bass_kernel_spmd(nc, in_maps, core_ids=list(range(N_CORES)))

    out = np.empty((B, T, C), dtype=np.float32)
    for b in range(B):
        pt = res.results[2 * b]["ot"] + res.results[2 * b + 1]["ot"]  # (C, T)
        out[b] = pt.T + out_b[None, :]
    return out



# revision 5
# speedup vs baseline: 1.2809x; 1.2809x over previous
"""Causal self-attention (B=4, T=2048, C=1024, H=16, D=64) on 8 TRN2 NeuronCores.

Sharding: core = (batch b, head-group g) with b = core // 2, g = core % 2.

v3 (stage 2): qkv/v projections run as fp8e4 DoubleRow matmuls (2x PE rate)
with a hi/lo residual split of both x and W prepared on the host:
  W.x ~= Whi.xhi + Wlo.xhi + Whi.xlo     (Wlo.xlo dropped, ~0.4% rms)
Each product is a K=256 DoubleRow accumulation step ([128, 2, N] APs, the
pair dim striding across two 128-row k-tiles). x is scaled by 4, W by 64;
the resulting 256x scale is folded into the rope tables / bias (qk path)
and cancels between the A@V numerator and the ones-column denominator
(v path). x lives in a t-major block layout [tch 16][kc 8][tl 128] so
phase-0 can start after the first t-chunk DMA lands.

Other structure as stage 1: transposed att@V (E stationary, N=65),
per-partition-denominator normalization, XBAR DMA-transpose of y,
tri-mask diagonal, fp16 output partials, Pool offload for psum copies.
"""

import numpy as np

B, T, C = 4, 2048, 1024
H, D = 16, 64
N_CORES = 8
HPG = H // 2            # heads per core (group)
NCHUNK = 4              # head-pair chunks per core
KT = 8                  # k-tiles of 128 over C
NPAIR = KT // 2         # DoubleRow k-tile pairs
TT = 4                  # t-tiles of 512 over T
NT = 512                # t tile (matmul N)
VS = 66                 # v column stride per head (64 dims + ones + pad)
VW = HPG * VS           # 528 v columns per k-chunk block
ROPE_BASE = 10000.0
XSC = 4.0               # fp8 scale for x
WSC = 64.0              # fp8 scale for weights
SSC = XSC * WSC         # combined psum scale (256)

_CACHE = {}


def _build_nc():
    import concourse.bass as bass  # noqa: F401
    import concourse.tile as tile
    from concourse import bacc, mybir
    from contextlib import ExitStack

    f16 = mybir.dt.float16
    f32 = mybir.dt.float32
    f8 = mybir.dt.float8e4
    DR = mybir.MatmulPerfMode.DoubleRow

    nc = bacc.Bacc(
        "TRN2",
        target_bir_lowering=False,
        debug=False,
        enable_asserts=True,
        num_devices=N_CORES,
    )

    xh_d = nc.dram_tensor("xh", (128, 16 * KT * 128), f8, kind="ExternalInput").ap()
    xl_d = nc.dram_tensor("xl", (128, 16 * KT * 128), f8, kind="ExternalInput").ap()
    wqkh_d = nc.dram_tensor("wqkh", (128, NCHUNK * 2048), f8, kind="ExternalInput").ap()
    wqkl_d = nc.dram_tensor("wqkl", (128, NCHUNK * 2048), f8, kind="ExternalInput").ap()
    wvh_d = nc.dram_tensor("wvh", (128, KT * VW), f8, kind="ExternalInput").ap()
    wvl_d = nc.dram_tensor("wvl", (128, KT * VW), f8, kind="ExternalInput").ap()
    wva_d = nc.dram_tensor("wva", (1, VW), f16, kind="ExternalInput").ap()
    wo_d = nc.dram_tensor("wo", (128, NCHUNK * 1024), f16, kind="ExternalInput").ap()
    bqk_d = nc.dram_tensor("bqk", (128, 16), f32, kind="ExternalInput").ap()
    cs_d = nc.dram_tensor("cs", (128, T), f16, kind="ExternalInput").ap()
    css_d = nc.dram_tensor("css", (128, T), f16, kind="ExternalInput").ap()
    tri_d = nc.dram_tensor("tri", (128, 128), f16, kind="ExternalInput").ap()
    ot_d = nc.dram_tensor("ot", (1024, T), f16, kind="ExternalOutput").ap()

    SHUF = list(range(16, 32)) + list(range(0, 16))

    with tile.TileContext(nc) as tc:
        with ExitStack() as ctx, nc.allow_low_precision("fp8/fp16 attention"):
            consts = ctx.enter_context(tc.tile_pool(name="consts", bufs=1))
            qk_pool = ctx.enter_context(tc.tile_pool(name="qk", bufs=2))
            rtmp = ctx.enter_context(tc.tile_pool(name="rtmp", bufs=4))
            e_pool = ctx.enter_context(tc.tile_pool(name="e", bufs=12))
            small = ctx.enter_context(tc.tile_pool(name="small", bufs=4))
            ytd_pool = ctx.enter_context(tc.tile_pool(name="ytd", bufs=3))
            osb = ctx.enter_context(tc.tile_pool(name="osb", bufs=2))
            pool_a = ctx.enter_context(tc.tile_pool(name="psa", bufs=2, space="PSUM"))
            pool_b = ctx.enter_context(tc.tile_pool(name="psb", bufs=2, space="PSUM"))
            pool_y = ctx.enter_context(tc.tile_pool(name="psy", bufs=2, space="PSUM"))

            # ---- resident tiles ----
            xh = consts.tile([128, 16 * KT * 128], f8)
            xl = consts.tile([128, 16 * KT * 128], f8)
            wqkh = consts.tile([128, NCHUNK * 2048], f8)
            wqkl = consts.tile([128, NCHUNK * 2048], f8)
            wvh = consts.tile([128, KT * VW], f8)
            wvl = consts.tile([128, KT * VW], f8)
            wva = consts.tile([1, VW], f16)
            wo = consts.tile([128, NCHUNK * 1024], f16)
            bqk = consts.tile([128, 16], f32)
            cs = consts.tile([128, T], f16)
            css = consts.tile([128, T], f16)
            tri = consts.tile([128, 128], f16)
            ones = consts.tile([1, 128], f16)
            v_sb = consts.tile([128, 16 * VW], f16)
            y_all = consts.tile([128, NCHUNK * T], f16)

            zro = consts.tile([1, 512], f16)
            nc.vector.memset(ones[:], 1.0)
            nc.vector.memset(zro[:], 0.0)

            # ---- input DMA, ordered by first use ----
            for i in range(2):  # x t-chunks 0..3 first (qkrope c0 tt0 + vproj m0..3)
                sl = slice(i * 2048, (i + 1) * 2048)
                nc.sync.dma_start(xh[:, sl], xh_d[:, sl])
                nc.sync.dma_start(xl[:, sl], xl_d[:, sl])
            nc.sync.dma_start(wqkh[:, 0:2048], wqkh_d[:, 0:2048])
            nc.sync.dma_start(wqkl[:, 0:2048], wqkl_d[:, 0:2048])
            nc.sync.dma_start(bqk[:], bqk_d[:])
            nc.sync.dma_start(cs[:, 0:1024], cs_d[:, 0:1024])
            nc.sync.dma_start(css[:, 0:1024], css_d[:, 0:1024])
            nc.sync.dma_start(tri[:], tri_d[:])
            nc.sync.dma_start(wvh[:], wvh_d[:])
            nc.sync.dma_start(wvl[:], wvl_d[:])
            nc.sync.dma_start(wva[:], wva_d[:])
            for i in range(2, 8):
                sl = slice(i * 2048, (i + 1) * 2048)
                nc.sync.dma_start(xh[:, sl], xh_d[:, sl])
                nc.sync.dma_start(xl[:, sl], xl_d[:, sl])
            for c in range(1, NCHUNK):
                sl = slice(c * 2048, (c + 1) * 2048)
                nc.sync.dma_start(wqkh[:, sl], wqkh_d[:, sl])
                nc.sync.dma_start(wqkl[:, sl], wqkl_d[:, sl])
            nc.sync.dma_start(cs[:, 1024:], cs_d[:, 1024:])
            nc.sync.dma_start(css[:, 1024:], css_d[:, 1024:])
            nc.sync.dma_start(wo[:], wo_d[:])

            # PE p-state warmup: harmless matmuls bridging the input-DMA
            # window so real matmuls start at full clock
            wup = pool_a.tile([128, 512], f32, tag="a")
            for _ in range(9):
                nc.tensor.matmul(wup[:], ones[:], zro[:], start=True, stop=True,
                                 skip_group_check=True)

            # 4D views: x as [p, kc, tch, tl]; wqk as [p, kc, which, feat]
            xh4 = xh[:].rearrange("p (tc k t) -> p k tc t", tc=16, k=KT)
            xl4 = xl[:].rearrange("p (tc k t) -> p k tc t", tc=16, k=KT)
            wqkh4 = wqkh[:].rearrange("p (c k w f) -> p c k w f", c=NCHUNK, k=KT, w=2)
            wqkl4 = wqkl[:].rearrange("p (c k w f) -> p c k w f", c=NCHUNK, k=KT, w=2)
            wvh3 = wvh[:].rearrange("p (k v) -> p k v", k=KT)
            wvl3 = wvl[:].rearrange("p (k v) -> p k v", k=KT)

            # ---- emission helpers (budget-scheduled fill units) ----
            # A "unit" is (est_pe_ns, fn). Attention pops units per-sc within
            # a budget so the exp stream stays dense while PE slack is filled.

            def vproj_units(m):
                st = {}
                prods = ((xh4, wvh3), (xl4, wvh3), (xh4, wvl3))

                def mk(i):
                    def u():
                        if i == 0:
                            st["psa"] = pool_a.tile([128, 512], f32, tag="a", name="vpsa")
                            st["psb"] = pool_y.tile([128, 260], f32, tag="y", name="vpsb")
                        lx, rw = prods[i]
                        for pp in range(NPAIR):
                            lhsT = lx[:, 2 * pp:2 * pp + 2, m, :]
                            rhs = rw[:, 2 * pp:2 * pp + 2, :]
                            stt = (i == 0 and pp == 0)
                            nc.tensor.matmul(st["psa"][:], lhsT, rhs[:, :, 0:512],
                                             start=stt, stop=False, perf_mode=DR)
                            nc.tensor.matmul(st["psb"][:, 0:16], lhsT, rhs[:, :, 512:VW],
                                             start=stt, stop=False, perf_mode=DR)
                    return u

                def fin():
                    nc.tensor.matmul(st["psa"][:], ones[:], wva[:, 0:512],
                                     start=False, stop=True)
                    nc.tensor.matmul(st["psb"][:, 0:16], ones[:], wva[:, 512:VW],
                                     start=False, stop=True)
                    nc.vector.tensor_copy(v_sb[:, m * VW: m * VW + 512], st["psa"][:])
                    nc.vector.tensor_copy(v_sb[:, m * VW + 512: (m + 1) * VW],
                                          st["psb"][:, 0:16])
                return [(450, mk(0)), (440, mk(1)), (440, mk(2)), (540, fin)]

            rqk = {}

            def qk_units(c, tt, which):
                st = {}
                prods = ((wqkh4, xh4), (wqkl4, xh4), (wqkh4, xl4))

                def mk(i):
                    def u():
                        if c not in rqk:
                            rq = qk_pool.tile([128, T], f16, tag="rq")
                            rk = qk_pool.tile([128, T], f16, tag="rk")
                            rqk[c] = (rq, rk)
                        if i == 0:
                            st["ps"] = pool_a.tile([128, 512], f32, tag="a", name="qkps")
                        lw, rx = prods[i]
                        for pp in range(NPAIR):
                            nc.tensor.matmul(
                                st["ps"][:],
                                lw[:, c, 2 * pp:2 * pp + 2, which, :],
                                rx[:, 2 * pp:2 * pp + 2, 4 * tt:4 * tt + 4, :],
                                start=(i == 0 and pp == 0),
                                stop=(i == 2 and pp == NPAIR - 1),
                                perf_mode=DR)
                    return u

                def rope():
                    t0 = tt * NT
                    dst = rqk[c][which]
                    ps = st["ps"]
                    bcol = bqk[:, c * 4 + which * 2: c * 4 + which * 2 + 1]
                    bswp = bqk[:, c * 4 + which * 2 + 1: c * 4 + which * 2 + 2]
                    s_t = rtmp.tile([128, 512], f32, tag="s")
                    nc.vector.stream_shuffle(s_t[:], ps[:], SHUF)
                    x1 = rtmp.tile([128, 512], f16, tag="x1")
                    nc.vector.scalar_tensor_tensor(
                        out=x1[:], in0=ps[:], scalar=bcol, in1=cs[:, t0:t0 + NT],
                        op0=mybir.AluOpType.add, op1=mybir.AluOpType.mult)
                    x2 = rtmp.tile([128, 512], f16, tag="x2")
                    nc.vector.scalar_tensor_tensor(
                        out=x2[:], in0=s_t[:], scalar=bswp, in1=css[:, t0:t0 + NT],
                        op0=mybir.AluOpType.add, op1=mybir.AluOpType.mult)
                    nc.vector.tensor_add(dst[:, t0:t0 + NT], x1[:], x2[:])
                return [(430, mk(0)), (430, mk(1)), (430, mk(2)), (60, rope)]

            otiles = {}

            def get_otile(tt):
                if tt not in otiles:
                    ot_sb = osb.tile([128, 4096], f16)
                    otiles[tt] = [ot_sb, 0]
                return otiles[tt]

            def oproj_unit(ct, tt, n_c=NCHUNK):
                def u():
                    t0 = tt * NT
                    po = pool_a.tile([128, 512], f32, tag="a")
                    for c in range(n_c):
                        nc.tensor.matmul(po[:], wo[:, c * 1024 + ct * 128: c * 1024 + ct * 128 + 128],
                                         y_all[:, c * T + t0: c * T + t0 + NT],
                                         start=(c == 0), stop=(c == n_c - 1))
                    ent = get_otile(tt)
                    nc.vector.tensor_copy(ent[0][:, ct * 512:(ct + 1) * 512], po[:])
                    ent[1] += 1
                    if ent[1] in (4, 8):
                        lo = 0 if ent[1] == 4 else 4
                        nc.sync.dma_start(
                            ot_d[lo * 128:(lo + 4) * 128, t0:t0 + NT]
                            .rearrange("(a p) t -> p a t", a=4),
                            ent[0][:, lo * 512:(lo + 4) * 512]
                            .rearrange("p (a t) -> p a t", a=4))
                return [(1070, u)]

            # tt3 split: accumulate c0..c2 into SBUF early; after the final
            # transpose only c3's matmul + an add remain per ct-tile
            opart = consts.tile([128, 4096], f16)

            def oproj_part_unit(ct):
                def u():
                    t3 = 3 * NT
                    po = pool_a.tile([128, 512], f32, tag="a")
                    for c in range(3):
                        nc.tensor.matmul(po[:], wo[:, c * 1024 + ct * 128: c * 1024 + ct * 128 + 128],
                                         y_all[:, c * T + t3: c * T + t3 + NT],
                                         start=(c == 0), stop=(c == 2))
                    nc.vector.tensor_copy(opart[:, ct * 512:(ct + 1) * 512], po[:])
                return [(1450, u)]

            def oproj_tail():
                t3 = 3 * NT
                ent = get_otile(3)
                for ct in range(8):
                    po = pool_a.tile([128, 512], f32, tag="a")
                    nc.tensor.matmul(po[:], wo[:, 3 * 1024 + ct * 128: 3 * 1024 + ct * 128 + 128],
                                     y_all[:, 3 * T + t3: 3 * T + t3 + NT],
                                     start=True, stop=True)
                    eng = nc.vector if ct % 2 == 0 else nc.gpsimd
                    eng.tensor_add(ent[0][:, ct * 512:(ct + 1) * 512],
                                   opart[:, ct * 512:(ct + 1) * 512], po[:])
                nc.sync.dma_start(
                    ot_d[:, t3:t3 + NT].rearrange("(a p) t -> p a t", a=8),
                    ent[0][:].rearrange("p (a t) -> p a t", a=8))

            def norm_tb(ytd, tb, yp, off):
                yc = small.tile([128, 130], f32, tag="yc")
                nc.vector.tensor_copy(yc[:], yp[:, off:off + 130])
                rd = small.tile([128, 2], f32, tag="rd")
                nc.vector.reciprocal(rd[:], yc[:, 64::65])
                for h in range(2):
                    nc.vector.tensor_scalar_mul(
                        ytd[:, tb * 128 + h * 64: tb * 128 + (h + 1) * 64],
                        yc[:, h * 65: h * 65 + 64],
                        rd[:, h:h + 1])

            def emit_attention(c, tt, fill, must_by=None, prev_tail=None):
                """Per-sc pipeline: score(sc), exp(sc), lag-1 av pumping of all
                four 128-t chains (two chains packed per psum bank). Fill units
                pop per-sc within a PE budget; must_by(sc) forces a minimum
                number of units (vproj data deadlines)."""
                rq, rk = rqk[c]
                t0 = tt * NT
                sc_max = (t0 + NT) // 128
                TBv = [4 * tt + i for i in range(4)]
                e_tiles = []
                ytd = ytd_pool.tile([128, 512], f16)
                yps = []
                popped = [0]

                def pop_unit():
                    cst, fn = fill.pop(0)
                    fn()
                    popped[0] += 1
                    return cst

                for half in range(2):
                    yp = pool_y.tile([128, 260], f32, tag="y")
                    # start=True clears has_written for the whole psum bank;
                    # one clear covers both packed chains
                    nc.tensor.matmul(yp[:], ones[:], zro[:, 0:260],
                                     start=True, stop=False,
                                     skip_group_check=True)
                    yps.append(yp)

                def av(tb, sc):
                    TB = TBv[tb]
                    yp = yps[tb // 2]
                    off = (tb % 2) * 130
                    e_t = e_tiles[sc]
                    for h in range(2):
                        nc.tensor.matmul(
                            yp[:, off + h * 65: off + (h + 1) * 65],
                            e_t[:, h * NT + tb * 128: h * NT + (tb + 1) * 128],
                            v_sb[:, sc * VW + VS * (2 * c + h): sc * VW + VS * (2 * c + h) + 65],
                            start=False,
                            stop=(sc == TB and h == 1 and tb % 2 == 1),
                            skip_group_check=True)

                def pump(p):
                    for tb in range(4):
                        if p <= TBv[tb]:
                            av(tb, p)
                            if p == TBv[tb]:
                                norm_tb(ytd, tb, yps[tb // 2], (tb % 2) * 130)

                for sc in range(sc_max):
                    s0 = sc * 128
                    dlt = max(0, s0 - t0)
                    sp = pool_b.tile([128, 1024], f32, tag="sp")
                    nc.tensor.matmul(sp[:, dlt:NT], rk[0:64, s0:s0 + 128],
                                     rq[0:64, t0 + dlt:t0 + NT],
                                     start=True, stop=True, tile_position=(0, 0))
                    nc.tensor.matmul(sp[:, NT + dlt:2 * NT], rk[64:128, s0:s0 + 128],
                                     rq[64:128, t0 + dlt:t0 + NT],
                                     start=True, stop=True, tile_position=(64, 0))
                    e_t = e_pool.tile([128, 1024], f16)
                    s3 = sp[:].rearrange("p (a b) -> p a b", a=2)[:, :, dlt:]
                    e3 = e_t[:].rearrange("p (a b) -> p a b", a=2)[:, :, dlt:]
                    nc.scalar.activation(e3, s3, mybir.ActivationFunctionType.Exp,
                                         bias=0.0, scale=0.125)
                    if s0 >= t0:
                        for h in range(2):
                            blk = e_t[:, h * NT + dlt: h * NT + dlt + 128]
                            nc.gpsimd.tensor_mul(blk, blk, tri[:])
                    e_tiles.append(e_t)
                    if sc == 1 and prev_tail is not None:
                        prev_tail()
                    if must_by is not None:
                        while fill and popped[0] < must_by(sc):
                            pop_unit()
                    bud = 500
                    while fill and bud > 0:
                        bud -= pop_unit()
                    if sc >= 2:
                        pump(sc - 2)

                def tail():
                    pump(sc_max - 2)
                    pump(sc_max - 1)
                    # y[t,d] -> y[d,t] per-128-block transpose into y_all
                    nc.sync.dma_start(
                        y_all[:, c * T + t0: c * T + t0 + NT].rearrange(
                            "p (b t) -> p b t", b=4),
                        ytd[:], transpose=True)
                return tail

            # ---- schedule ----
            # chunk 0: per tt, fill = this tt's 4 vproj m-slices (forced by
            # pump deadlines) + next chunk's qkrope for the same tt
            emit_qkrope = lambda c, tt: None  # (units only)
            prev_tail = None
            for tt in range(TT):
                for w in range(2):
                    for cst, fn in qk_units(0, tt, w):
                        fn()
                fill = []
                for k in range(4):
                    fill += vproj_units(4 * tt + k)
                fill += qk_units(1, tt, 0) + qk_units(1, tt, 1)

                def must(sc, tt=tt):
                    # vproj m=4tt+k (units 4k+1..4k+4) must be emitted before
                    # pump(p=m), which runs at sc=m-4tt+2 under lag-2
                    return min(16, 4 * (sc + 1))
                prev_tail = emit_attention(0, tt, fill, must_by=must,
                                           prev_tail=prev_tail)
                while fill:
                    fill.pop(0)[1]()
            # chunks 1..2: fill = next chunk's qkrope, carried across tts
            for c in (1, 2):
                fill = []
                for tt in range(TT):
                    fill += qk_units(c + 1, tt, 0) + qk_units(c + 1, tt, 1)
                    prev_tail = emit_attention(c, tt, fill, prev_tail=prev_tail)
                while fill:
                    fill.pop(0)[1]()
            # chunk 3: fill = output projection for completed t-tiles
            fill = []
            for tt in range(TT):
                t = emit_attention(3, tt, fill, prev_tail=prev_tail)
                if tt < 3:
                    def wrapped(t=t, tt=tt):
                        t()
                        for ct in range(8):
                            fill.extend(oproj_unit(ct, tt))
                    prev_tail = wrapped
                else:
                    prev_tail = t
            prev_tail()
            while fill:
                fill.pop(0)[1]()
            for ct in range(8):
                for cst, fn in oproj_unit(ct, 3):
                    fn()

    nc.compile()
    return nc


def _fp8_hilo(a):
    """Split array a into e4m3 hi + residual lo (same scale)."""
    import ml_dtypes
    hi = a.astype(ml_dtypes.float8_e4m3)
    lo = (a - hi.astype(np.float32)).astype(ml_dtypes.float8_e4m3)
    return hi, lo


def _prep_inputs(x, qkv_w, qkv_b):
    """Build the 8 per-core input maps (all host-side numpy)."""
    x = np.asarray(x, dtype=np.float32)
    qkv_w = np.asarray(qkv_w, dtype=np.float32)
    qkv_b = np.asarray(qkv_b, dtype=np.float32)

    # x in t-major block layout [p, tch*1024 + kc*128 + tl], scaled by XSC
    xhs, xls = [], []
    for b in range(B):
        x4 = (XSC * x[b].T).astype(np.float32)          # (C, T)
        blk = x4.reshape(KT, 128, 16, 128).transpose(1, 2, 0, 3).reshape(128, 16 * KT * 128)
        hi, lo = _fp8_hilo(blk)
        xhs.append(hi)
        xls.append(lo)

    r = np.arange(64)
    d_r = 2 * ((r // 32) * 16 + (r % 16)) + ((r % 32) >= 16)  # row -> head dim
    p = np.arange(128)
    f_p = ((p // 32) % 2) * 16 + (p % 16)

    ins_g = []
    for g in range(2):
        # wqk fp8 hi/lo: [p, c*2048 + kc*256 + which*128 + m], scaled by WSC
        wqkh = np.empty((128, NCHUNK * 2048), dtype=np.float32)
        bqk = np.empty((128, 16), dtype=np.float32)
        for c in range(NCHUNK):
            for which in range(2):  # 0=q, 1=k
                rows = np.concatenate([
                    which * C + (8 * g + 2 * c + hh) * 64 + d_r for hh in range(2)
                ])  # 128 feature rows
                blk = (WSC * qkv_w[rows, :]).T          # (1024 k, 128 feat)
                for kc in range(KT):
                    wqkh[:, c * 2048 + kc * 256 + which * 128:
                         c * 2048 + kc * 256 + which * 128 + 128] = \
                        blk[kc * 128:(kc + 1) * 128, :]
                bc = SSC * qkv_b[rows].astype(np.float32)
                bqk[:, c * 4 + which * 2] = bc
                bqk[:, c * 4 + which * 2 + 1] = bc[p ^ 16]
        wqk_hi, wqk_lo = _fp8_hilo(wqkh)
        # wv fp8 hi/lo: [p, kc*VW + col], col = VS*h + j; aug row fp16 * SSC
        wva = np.zeros((C, VW), dtype=np.float32)
        aug = np.zeros((1, VW), dtype=np.float32)
        for h in range(HPG):
            rows = 2 * C + (8 * g + h) * 64 + np.arange(64)
            wva[:, VS * h: VS * h + 64] = WSC * qkv_w[rows, :].T
            aug[0, VS * h: VS * h + 64] = SSC * qkv_b[rows]
            aug[0, VS * h + 64] = SSC
        wv = wva.reshape(KT, 128, VW).transpose(1, 0, 2).reshape(128, KT * VW)
        wv_hi, wv_lo = _fp8_hilo(wv)
        ins_g.append((wqk_hi, wqk_lo, bqk, wv_hi, wv_lo, aug.astype(np.float16)))

    # rope tables, folded 1/SSC
    inv_freq = (1.0 / (ROPE_BASE ** (np.arange(0, D, 2) / D))).astype(np.float64)
    t = np.arange(T, dtype=np.float64)
    ang = t[None, :] * inv_freq[f_p][:, None]          # (128, T)
    cs = (np.cos(ang) / SSC).astype(np.float16)
    sgn = np.where((p % 32) < 16, -1.0, 1.0)[:, None]
    css = (sgn * np.sin(ang) / SSC).astype(np.float16)

    # upper-triangular (keep t >= s) mask tile
    tri = (np.arange(128)[None, :] >= np.arange(128)[:, None]).astype(np.float16)

    return xhs, xls, ins_g, cs, css, tri


def _prep_wo(out_w, g):
    out_w = np.asarray(out_w, dtype=np.float32)
    wo = np.empty((128, NCHUNK * 1024), dtype=np.float16)
    for c in range(NCHUNK):
        rows = np.concatenate([(8 * g + 2 * c + hh) * 64 + np.arange(64) for hh in range(2)])
        wo[:, c * 1024:(c + 1) * 1024] = out_w[:, rows].astype(np.float16).T
    return wo


def kernel(x, qkv_w, qkv_b, out_w, out_b):
    from concourse.bass_utils import run_bass_kernel_spmd

    if "nc" not in _CACHE:
        _CACHE["nc"] = _build_nc()
    nc = _CACHE["nc"]

    xhs, xls, ins_g, cs, css, tri = _prep_inputs(x, qkv_w, qkv_b)
    wos = [_prep_wo(out_w, g) for g in range(2)]
    out_b = np.asarray(out_b, dtype=np.float32)

    in_maps = []
    for core in range(N_CORES):
        b, g = core // 2, core % 2
        wqk_hi, wqk_lo, bqk, wv_hi, wv_lo, aug = ins_g[g]
        in_maps.append({
            "xh": xhs[b], "xl": xls[b],
            "wqkh": wqk_hi, "wqkl": wqk_lo, "wvh": wv_hi, "wvl": wv_lo,
            "wva": aug, "wo": wos[g],
            "bqk": bqk, "cs": cs, "css": css, "tri": tri,
        })

    try:
        res = run_bass_kernel_spmd(nc, in_maps, core_ids=list(range(N_CORES)))
    except ModuleNotFoundError:
        # BASS_TRACE set but the NTFF profile hook isn't importable here
        import os
        os.environ["BASS_NEVER_TRACE"] = "1"
        res = run_bass_kernel_spmd(nc, in_maps, core_ids=list(range(N_CORES)))

    out = np.empty((B, T, C), dtype=np.float32)
    for b in range(B):
        pt = res.results[2 * b]["ot"].astype(np.float32) + \
            res.results[2 * b + 1]["ot"].astype(np.float32)  # (C, T)
        out[b] = pt.T + out_b[None, :]
    return out
